# revision 43
# baseline (speedup 1.0000x reference)
"""Causal self-attention (B=4, T=2048, C=1024, H=16, D=64) on 8 TRN2 NeuronCores.

Sharding: 8 cores = 4 batches x 2 head-groups (8 heads each). Each core:
  - QKV projection for its (batch, head-group) column slice of w_attn,
    producing qT/kT in [d, t] layout (transposed dataflow) and v in [t, d].
  - Causal attention in scoresT layout (scores^T[k, q] comes straight out of
    the PE; softmax denominators via an appended ones-column on V; no PE
    transposes anywhere).
  - Row-sharded output projection -> per-core bf16 partial [T, C].
Host sums the two partials per batch in fp32 and adds b_proj. The 1/sqrt(D)
attention scale is folded into the Q weights (and Q bias) on the host, so
the exp activation runs scale-free.

Matmul operands are bf16 (1 col/cycle on the PE; K=64 matmuls clock slower
on TRN2, hence the zero-padded K=128 QK tiles) with fp32 PSUM accumulation.
Schedule notes (all aimed at keeping the PE saturated -- TRN2's PE p-state
throttle makes every idle gap cost extra ramp time):
  - Zero-bias fast path: the K=1 bias matmuls are only emitted when b_attn
    is nonzero (checked on host; separate cached build per case).
  - One continuous QK->exp->AV pipeline across all heads with a 2-step
    QK lead over AV: the exp of one step is slower than its two matmuls, so
    AV_j only issues after QK_{j+2}, and the pend queue never flushes at a
    head boundary.
  - Softmax normalize: DVE reciprocal on [1,512] + GpSimd partition
    broadcast + one DVE multiply (no fp32 PE broadcast matmuls). A head's
    normalize drains attached to its last AV, i.e. after the next head's
    first QKs are already queued.
  - Filler scattering: projection work that needs no ACT (quarters 2-3 of
    the QKV projection, split into 4-matmul halves, and 512-channel chunks
    of the output projection) is emitted inside the attention j-loops,
    right before AV steps, sized so ACT's exp never starves the PE. Fillers
    precede each step's causal mask so their PSUM evacuations clear the DVE
    queue ahead of mask ops that may stall on ACT.
  - PSUM: 3x [128,1024] score tiles (shared rotation with projection
    accumulators) + 2x [65,512] y tiles fill all 8 banks.
  - DMA: all activation/weight loads interleave across the two HWDGE
    queues (sync/scalar) with xq column halves landing independently.
    The gpsimd SWDGE queue carries ONLY the normalize broadcasts plus the
    startup memsets: any bulk transfer queued there delays a broadcast,
    which stalls the y-PSUM rotation and the PE. Output partials leave as
    bf16 on sync (scalar joins in the ACT-idle tail).
"""

import sys
import types

import numpy as np

B, T, C, H, D = 4, 2048, 1024, 16, 64
HG = 8            # heads per core
CG = HG * D       # 512 channels per group
NCORES = 8
TB = T // 128     # 16 t-blocks
QCH = T // 512    # 4 t-quarters


def _register_ntff_hook():
    """Register the axon NTFF profile hook if the image's antenv lacks it."""
    try:
        import antenv
        if getattr(antenv, "axon_hooks", None) is not None:
            return
        from trn_agent_boot.trn_boot import _ntff_profile_via_ctypes
        hook = _ntff_profile_via_ctypes("/opt/axon/libaxon_pjrt.so")
        mod = types.ModuleType("antenv.axon_hooks")
        mod._hook = hook
        mod.get_axon_ntff_profile_hook = lambda: mod._hook
        mod.set_axon_ntff_profile_hook = lambda h: setattr(mod, "_hook", h)
        sys.modules["antenv.axon_hooks"] = mod
        antenv.axon_hooks = mod
    except Exception:
        pass


_NC_CACHE = {}


def _build(with_bias):
    import concourse.bacc as bacc
    import concourse.mybir as mybir
    import concourse.tile as tile
    from concourse.masks import make_upper_triangular
    from contextlib import ExitStack

    F32 = mybir.dt.float32
    BF16 = mybir.dt.bfloat16
    MUL = mybir.AluOpType.mult
    EXP = mybir.ActivationFunctionType.Exp

    nc = bacc.Bacc(None, target_bir_lowering=False, debug=False)
    xT_d = nc.dram_tensor("xT", [C, T], BF16, kind="ExternalInput")
    wqk_d = nc.dram_tensor("wqk", [C, 2 * CG], BF16, kind="ExternalInput")
    wv_d = nc.dram_tensor("wv", [C, CG], BF16, kind="ExternalInput")
    wp_d = nc.dram_tensor("wp", [CG, C], BF16, kind="ExternalInput")
    if with_bias:
        bqk_d = nc.dram_tensor("bqk", [1, 2 * CG], BF16, kind="ExternalInput")
        bv_d = nc.dram_tensor("bv", [1, CG], BF16, kind="ExternalInput")
    out_d = nc.dram_tensor("out", [T, C], BF16, kind="ExternalOutput")

    CT = C // 128  # 8 c-tiles of the contraction dim

    with tile.TileContext(nc) as tc, ExitStack() as ctx:
        pers = ctx.enter_context(tc.tile_pool(name="pers", bufs=1))

        # Per-head qT/kT tiles in [d, t] layout. Head h's 64 d-rows live at
        # partitions (h%2)*64..(h%2)*64+64 (matching the projection PSUM
        # layout); the other 64 partitions are zeroed so QK matmuls contract
        # over a full K=128 (zeros add nothing; K=64 matmuls run at a
        # reduced PE clock on TRN2, so the padding is a win).
        qTt = [pers.tile([128, T], BF16, name=f"qTt{h}") for h in range(HG)]
        kTt = [pers.tile([128, T], BF16, name=f"kTt{h}") for h in range(HG)]
        # v_aug[p, j, h, 0:64] = v[t=j*128+p, h*64+d]; [..., 64] = 1.0
        v_aug = pers.tile([128, TB, HG, 65], BF16, name="v_aug")
        utri = pers.tile([128, 128], BF16, name="utri")
        ones_q = pers.tile([1, 512], BF16, name="ones_q")
        if with_bias:
            bqk_sb = pers.tile([1, 2 * CG], BF16, name="bqk_sb")
            bv_sb = pers.tile([1, CG], BF16, name="bv_sb")
        stage = pers.tile([128, 512], F32, name="stage")

        wp_pool = ctx.enter_context(tc.tile_pool(name="wp_pool", bufs=1))
        wp_sb = [wp_pool.tile([128, C], BF16, name=f"wp{i}") for i in range(4)]
        yT_pool = ctx.enter_context(tc.tile_pool(name="yT_pool", bufs=1))
        yT = [yT_pool.tile([128, T], BF16, name=f"yT{i}") for i in range(4)]

        att_pool = ctx.enter_context(tc.tile_pool(name="att_pool", bufs=10))
        nrm_pool = ctx.enter_context(tc.tile_pool(name="nrm_pool", bufs=4))
        out_pool = ctx.enter_context(tc.tile_pool(name="out_pool", bufs=3))
        ps_s_pool = ctx.enter_context(
            tc.tile_pool(name="ps_s_pool", bufs=3, space="PSUM"))
        ps_y_pool = ctx.enter_context(
            tc.tile_pool(name="ps_y_pool", bufs=2, space="PSUM"))

        # Phase-1 working pools (manually released once quarters 2-3 finish).
        wqk_pool = tc.alloc_tile_pool(name="wqk_pool", bufs=1)
        wv_pool = tc.alloc_tile_pool(name="wv_pool", bufs=1)
        xq_pool = tc.alloc_tile_pool(name="xq_pool", bufs=2)
        wqk_sb = [wqk_pool.tile([128, 2 * CG], BF16, name=f"wqk{c}")
                  for c in range(CT)]
        wv_sb = [wv_pool.tile([128, CG], BF16, name=f"wv{c}") for c in range(CT)]

        # Four distinct engine trigger queues so the startup load and the
        # steady-state output traffic aren't serialized on one queue.
        dma_engines = [nc.sync, nc.gpsimd, nc.scalar]

        xq_by_q = {}

        def p1_dma(q):
            # Column halves land independently (subtile deps) so the first
            # t-blocks of a quarter unblock before the whole quarter arrives.
            # Startup quarters ride the two fast HWDGE queues; later quarters
            # also use the gpsimd SWDGE queue.
            qs = [nc.sync, nc.scalar]
            nq = len(qs)
            xq = []
            for c in range(CT):
                xt = xq_pool.tile([128, 512], BF16, name=f"xq{c}", tag=f"xq{c}")
                for half in range(2):
                    qs[(c + half) % nq if nq == 2 else (2 * c + half) % nq].dma_start(
                        xt[:, half * 256:(half + 1) * 256],
                        xT_d.ap()[c * 128:(c + 1) * 128,
                                  q * 512 + half * 256:q * 512 + (half + 1) * 256])
                xq.append(xt)
            xq_by_q[q] = xq

        # ---- startup DMA triggers first (transfers overlap the constant
        # staging below) ----
        hwq = [nc.sync, nc.scalar]
        # Interleave wv with the first xq halves so the very first V matmuls
        # (which consume (wv[c], xq0[c]) pairs in c order) can start while
        # the rest of the quarter is still in flight.
        xq0 = [xq_pool.tile([128, 512], BF16, name=f"xq{c}", tag=f"xq{c}")
               for c in range(CT)]
        for c in range(CT):
            hwq[c % 2].dma_start(
                wv_sb[c][:], wv_d.ap()[c * 128:(c + 1) * 128, :])
            hwq[(c + 1) % 2].dma_start(
                xq0[c][:, 0:256], xT_d.ap()[c * 128:(c + 1) * 128, 0:256])
        for c in range(CT):
            hwq[c % 2].dma_start(
                xq0[c][:, 256:512], xT_d.ap()[c * 128:(c + 1) * 128, 256:512])
        xq_by_q[0] = xq0
        for c in range(CT):
            hwq[c % 2].dma_start(
                wqk_sb[c][:], wqk_d.ap()[c * 128:(c + 1) * 128, :])
        p1_dma(1)
        if with_bias:
            nc.sync.dma_start(bqk_sb[:], bqk_d.ap()[:])
            nc.sync.dma_start(bv_sb[:], bv_d.ap()[:])

        # ---- constants ----
        for h in range(HG):
            zs = slice(64, 128) if h % 2 == 0 else slice(0, 64)
            nc.gpsimd.memset(qTt[h][zs, :], 0.0)
            nc.gpsimd.memset(kTt[h][zs, :], 0.0)
        make_upper_triangular(nc, utri[:, :], val=1.0, diag=True)
        nc.vector.memset(stage[:], 1.0)
        nc.vector.tensor_copy(ones_q[:], stage[0:1, :])
        nc.vector.tensor_copy(
            v_aug[:, :, :, 64:65],
            stage[:, 0:128].rearrange("p (j h) -> p j h", j=TB))

        def v_unit_halves(q, tb):
            """V projection for t-block tb of quarter q, as two closures.

            The halves must be emitted within two attention QK steps of each
            other: the accumulating PSUM tile stays open between them and
            the 3-deep ps_s rotation would otherwise hand its bank to a
            later QK before the evacuation is even emitted.
            """
            st = {}

            def a():
                st["pv"] = ps_s_pool.tile([128, CG], F32, name="pv", tag="ps_s")
                for c in range(4):
                    nc.tensor.matmul(
                        st["pv"][:], xq_by_q[q][c][:, tb * 128:(tb + 1) * 128],
                        wv_sb[c][:], start=(c == 0), stop=False)

            def b():
                pv = st["pv"]
                for c in range(4, CT):
                    nc.tensor.matmul(
                        pv[:], xq_by_q[q][c][:, tb * 128:(tb + 1) * 128],
                        wv_sb[c][:], start=False,
                        stop=(not with_bias and c == CT - 1))
                if with_bias:
                    nc.tensor.matmul(
                        pv[:], ones_q[:, tb * 128:(tb + 1) * 128], bv_sb[:],
                        start=False, stop=True)
                j = q * 4 + tb
                nc.vector.tensor_copy(
                    v_aug[:, j, :, 0:64],
                    pv[:].rearrange("p (h d) -> p h d", h=HG))

            return [a, b]

        def qk_unit_halves(q, m):
            """Q/K projection M-block m of quarter q, as two closures."""
            st = {}

            def a():
                st["p"] = ps_s_pool.tile([128, 512], F32, name="pqk",
                                         tag="ps_s")
                for c in range(4):
                    nc.tensor.matmul(
                        st["p"][:], wqk_sb[c][:, m * 128:(m + 1) * 128],
                        xq_by_q[q][c][:], start=(c == 0), stop=False)

            def b():
                pqk = st["p"]
                for c in range(4, CT):
                    nc.tensor.matmul(
                        pqk[:], wqk_sb[c][:, m * 128:(m + 1) * 128],
                        xq_by_q[q][c][:], start=False,
                        stop=(not with_bias and c == CT - 1))
                if with_bias:
                    nc.tensor.matmul(
                        pqk[:], bqk_sb[:, m * 128:(m + 1) * 128], ones_q[:],
                        start=False, stop=True)
                dst = qTt if m < 4 else kTt
                h0 = 2 * (m % 4)
                sl = slice(q * 512, (q + 1) * 512)
                nc.vector.tensor_copy(dst[h0][0:64, sl], pqk[0:64, :])
                # Quarters 0/1 run before any attention: ACT is idle there,
                # so it takes half the evacuation load off the DVE queue
                # that the score-tile rotation waits on. Quarters 2/3 run
                # mid-attention where ACT paces the exps -- keep those on
                # DVE.
                if q < 2:
                    nc.scalar.copy(dst[h0 + 1][64:128, sl], pqk[64:128, :])
                else:
                    nc.vector.tensor_copy(
                        dst[h0 + 1][64:128, sl], pqk[64:128, :])

            return [a, b]

        def p1_v_unit(q, tb):
            for f in v_unit_halves(q, tb):
                f()

        def p1_qk_unit(q, m):
            for f in qk_unit_halves(q, m):
                f()

        def p1_units(q):
            for tb in range(4):
                yield lambda tb=tb: p1_v_unit(q, tb)
            for m in range(8):
                yield lambda m=m: p1_qk_unit(q, m)

        def normalize_pair(ps_y0, ps_y1, h, c2):
            """yT[d, q] /= sums[q] for both halves of a head.

            The DVE chain is interleaved across the two PSUM tiles: both
            reciprocals are issued before the first multiply, so the gpsimd
            partition_broadcast latency hides behind DVE work instead of
            head-of-line blocking the DVE queue (which carries the next
            head's causal masks).
            """
            bcs = []
            for ps_y in (ps_y0, ps_y1):
                # Cross-partition move (64 -> 0) must be a plain copy; DVE
                # arithmetic ops are partition-locked.
                sums_sb = nrm_pool.tile([1, 512], F32, tag="sums")
                nc.vector.tensor_copy(sums_sb[:], ps_y[64:65, :])
                inv_sb = nrm_pool.tile([1, 512], F32, tag="inv")
                nc.vector.reciprocal_approx_fast(inv_sb[:], sums_sb[:])
                bc_sb = nrm_pool.tile([64, 512], F32, tag="bc")
                nc.gpsimd.partition_broadcast(bc_sb[:], inv_sb[:])
                bcs.append(bc_sb)
            for i, ps_y in enumerate((ps_y0, ps_y1)):
                cch = 2 * c2 + i
                ct, sl = h // 2, slice(cch * 512, (cch + 1) * 512)
                if h % 2 == 0:
                    nc.vector.tensor_tensor(
                        out=yT[ct][0:64, sl], in0=ps_y[0:64, :],
                        in1=bcs[i][:], op=MUL)
                else:
                    ystg = nrm_pool.tile([64, 512], BF16, tag="ystg")
                    nc.vector.tensor_tensor(
                        out=ystg[:], in0=ps_y[0:64, :], in1=bcs[i][:], op=MUL)
                    nc.sync.dma_start(yT[ct][64:128, sl], ystg[:])

        def qk_exp(h, c2, j):
            """QK matmuls + exp + causal mask for one (head, superchunk, j).

            Scores go through single-bank [128,512] PSUM tiles with a
            per-half exp so AV's first half only waits on the first exp.
            Returns (att, dead) for the later AV step.
            """
            q0 = c2 * 1024
            dead = (j - 8 * c2) * 128 if j >= 8 * c2 else 0
            att = att_pool.tile([128, 1024], BF16, tag="att")
            kblk = kTt[h][:, j * 128:(j + 1) * 128]
            ps_s = ps_s_pool.tile([128, 1024], F32, name="ps_s", tag="ps_s")
            if dead < 512:
                nc.tensor.matmul(
                    ps_s[:, dead:512], kblk, qTt[h][:, q0 + dead:q0 + 512],
                    start=True, stop=True)
            lo_s = max(512, dead)
            nc.tensor.matmul(
                ps_s[:, lo_s:1024], kblk, qTt[h][:, q0 + lo_s:q0 + 1024],
                start=True, stop=True)
            nc.scalar.activation(
                att[:, dead:1024], ps_s[:, dead:1024], EXP)
            if j >= 8 * c2:
                nc.vector.tensor_tensor(
                    out=att[:, dead:dead + 128], in0=att[:, dead:dead + 128],
                    in1=utri[:, :], op=MUL)
            return att, dead

        def av_step(h, c2, j, att, dead, ps_y0, ps_y1):
            jmax = 8 * c2 + 7
            if j <= 8 * c2 + 3:
                nc.tensor.matmul(
                    ps_y0[:, dead:512], v_aug[:, j, h, :], att[:, dead:512],
                    start=(j == 0), stop=(j == 8 * c2 + 3))
            lo1 = max(512, dead)
            nc.tensor.matmul(
                ps_y1[:, lo1 - 512:512], v_aug[:, j, h, :], att[:, lo1:1024],
                start=(j == 0), stop=(j == jmax))

        # One continuous QK->exp->AV pipeline across ALL heads: the pend
        # queue never flushes at a head boundary, so the PE always has the
        # next head's QK work in front of a pending AV while ACT drains its
        # exp backlog. A head's normalize_pair is attached to its last AV
        # and emitted right when that AV drains -- after the next head's
        # first QKs, but before the next head's first AV write (so the
        # PSUM pool's write-after-read tracking stays sound).
        attn_pend = []

        def attn_push(avfn, normfn=None):
            attn_pend.append((avfn, normfn))
            if len(attn_pend) > 2:
                fn, nf = attn_pend.pop(0)
                fn()
                if nf is not None:
                    nf()

        def attn_flush():
            while attn_pend:
                fn, nf = attn_pend.pop(0)
                fn()
                if nf is not None:
                    nf()

        def attn_head(h, c2, fillers=()):
            # Filler units (projection work that needs no ACT) are emitted
            # right before AV steps so the PE chews on them during exp
            # backlog instead of idling.
            fillers = list(fillers)
            ps_y0 = ps_y_pool.tile([65, 512], F32, name="ps_y0", tag="ps_y")
            ps_y1 = ps_y_pool.tile([65, 512], F32, name="ps_y1", tag="ps_y")
            jlast = 8 * c2 + 7
            for j in range(jlast + 1):
                # Fillers go first: their PSUM evacuations enter the DVE
                # queue ahead of this step's causal mask, which may stall
                # on ACT's exp -- evacuations don't depend on exp, and the
                # ps_s rotation needs them retired promptly.
                while fillers and fillers[0][0] <= j:
                    fillers.pop(0)[1]()
                att, dead = qk_exp(h, c2, j)
                nf = (lambda: normalize_pair(ps_y0, ps_y1, h, c2)) \
                    if j == jlast else None
                attn_push(
                    lambda j=j, att=att, dead=dead:
                        av_step(h, c2, j, att, dead, ps_y0, ps_y1),
                    nf)
            for _, f in fillers:
                f()

        def proj_ch(tb, ch, state, tail=False):
            if "o" not in state:
                state["o"] = out_pool.tile([128, C], BF16, name="o_sb",
                                           tag="o_sb")
            o_sb = state["o"]
            pp = ps_s_pool.tile([128, 512], F32, name="pp", tag="ps_s")
            for ct in range(4):
                nc.tensor.matmul(
                    pp[:],
                    yT[ct][:, tb * 128:(tb + 1) * 128],
                    wp_sb[ct][:, ch * 512:(ch + 1) * 512],
                    start=(ct == 0), stop=(ct == 3))
            # In the ACT-idle tail, odd-channel evacuations ride the scalar
            # engine so the PSUM rotation never waits on the DVE queue.
            if tail and ch == 1:
                nc.scalar.copy(o_sb[:, ch * 512:(ch + 1) * 512], pp[:])
            else:
                nc.vector.tensor_copy(o_sb[:, ch * 512:(ch + 1) * 512], pp[:])
            if ch == 1:
                # In the tail ACT is idle, so the second half rides the
                # scalar HWDGE queue; mid-attention it takes the gpsimd
                # SWDGE queue to stay clear of the exp-busy scalar queue.
                eng2 = nc.scalar if tail else nc.sync
                nc.sync.dma_start(
                    out_d.ap()[tb * 128:(tb + 1) * 128, 0:512], o_sb[:, 0:512])
                eng2.dma_start(
                    out_d.ap()[tb * 128:(tb + 1) * 128, 512:1024],
                    o_sb[:, 512:1024])

        def proj_unit(tb):
            state = {}
            proj_ch(tb, 0, state, tail=True)
            proj_ch(tb, 1, state, tail=True)

        # ---------------- Orchestration ----------------
        # Quarters 0-1 straight through.
        for u in p1_units(0):
            u()
        for u in p1_units(1):
            u()
        for i in range(4):
            hwq[i % 2].dma_start(
                wp_sb[i][:], wp_d.ap()[i * 128:(i + 1) * 128, :])

        # Attention on q < 1024 interleaved with projection quarters 2-3.
        # Odd heads first: their yT writes go through a staging DMA (cross-
        # partition move), so give them slack; the last head's direct DVE
        # write then gates the downstream projection with minimal latency.
        #
        # V units of q2/q3 (needed by every c2=1 AV at j>=8) and the q-side
        # QK-projection units (m 0-3) interleave into the c2=0 loop; the
        # k-side units (m 4-7, only needed by a head's own j>=8 QK) move
        # into the c2=1 loop as filler there.
        horder = [1, 3, 5, 7, 0, 2, 4, 6]
        p1_dma(2)
        f0 = [("v", 2, tb) for tb in range(4)] + [("qk", 2, m) for m in range(4)]
        f0b = [("v", 3, tb) for tb in range(4)] + [("qk", 3, m) for m in range(4)]

        def halves_of(u):
            kind, q, i = u
            return v_unit_halves(q, i) if kind == "v" else qk_unit_halves(q, i)

        for i, h in enumerate(horder):
            if i == 0:
                u1 = halves_of(f0[0])
                attn_head(h, 0, [(1, u1[0]), (3, u1[1])])
                p1_dma(3)
                fill = f0[1:] + f0b
            else:
                take, fill = fill[:2], fill[2:]
                u1, u2 = halves_of(take[0]), halves_of(take[1])
                attn_head(h, 0, [(1, u1[0]), (3, u1[1]),
                                 (5, u2[0]), (7, u2[1])])
        for u in fill:
            kind, q, i = u
            (p1_v_unit if kind == "v" else p1_qk_unit)(q, i)

        # Attention on q >= 1024. Odd heads carry their own k-side QK
        # projection units as filler (must land before their j=8); even
        # heads carry the ready half of the output projection (t < 1024
        # only needs yT chunks 0-1), one 512-channel chunk at a time.
        for i, h in enumerate(horder):
            if h % 2 == 1:
                m = 4 + h // 2
                u1, u2 = qk_unit_halves(2, m), qk_unit_halves(3, m)
                fl = [(1, u1[0]), (3, u1[1]), (6, u2[0]), (8, u2[1])]
            else:
                tb0 = (i - 4) * 2
                st0, st1 = {}, {}
                fl = [(1, lambda: proj_ch(tb0, 0, st0)),
                      (4, lambda: proj_ch(tb0, 1, st0)),
                      (8, lambda: proj_ch(tb0 + 1, 0, st1)),
                      (12, lambda: proj_ch(tb0 + 1, 1, st1))]
            attn_head(h, 1, fl)
            if i == len(horder) - 1:
                xq_pool.release()
                wv_pool.release()
                wqk_pool.release()
        attn_flush()
        for tb in range(8, 16):
            proj_unit(tb)

    nc.compile()
    return nc


def _get_nc(with_bias):
    key = ("nc", with_bias)
    if key not in _NC_CACHE:
        _register_ntff_hook()
        _NC_CACHE[key] = _build(with_bias)
    return _NC_CACHE[key]


def kernel(x, w_attn, b_attn, w_proj, b_proj, _run_kwargs=None):
    import ml_dtypes
    from concourse.bass_utils import run_bass_kernel_spmd

    bf16 = ml_dtypes.bfloat16
    x = np.asarray(x, dtype=np.float32)
    w_attn = np.asarray(w_attn, dtype=np.float32)
    b_attn = np.asarray(b_attn, dtype=np.float32)
    w_proj = np.asarray(w_proj, dtype=np.float32)
    b_proj = np.asarray(b_proj, dtype=np.float32)

    with_bias = bool(np.any(b_attn))
    nc = _get_nc(with_bias)
    in_maps = []
    for core in range(NCORES):
        b, g = divmod(core, 2)
        cols = slice(g * CG, (g + 1) * CG)
        im = {
            "xT": np.ascontiguousarray(x[b].T).astype(bf16),
            # 1/sqrt(D) folded into the Q projection so the exp activation
            # runs scale-free.
            "wqk": np.concatenate(
                [w_attn[:, cols] * np.float32(0.125),
                 w_attn[:, C + g * CG: C + (g + 1) * CG]],
                axis=1).astype(bf16),
            "wv": np.ascontiguousarray(
                w_attn[:, 2 * C + g * CG: 2 * C + (g + 1) * CG]).astype(bf16),
            "wp": np.ascontiguousarray(w_proj[g * CG:(g + 1) * CG, :]).astype(bf16),
        }
        if with_bias:
            # Q bias carries the same folded 1/sqrt(D) as the Q weights.
            im["bqk"] = np.concatenate(
                [b_attn[cols] * np.float32(0.125),
                 b_attn[C + g * CG: C + (g + 1) * CG]]
            ).reshape(1, -1).astype(bf16)
            im["bv"] = np.ascontiguousarray(
                b_attn[2 * C + g * CG: 2 * C + (g + 1) * CG]).reshape(1, -1).astype(bf16)
        in_maps.append(im)

    res = run_bass_kernel_spmd(nc, in_maps, core_ids=list(range(NCORES)),
                               **(_run_kwargs or {}))
    out = np.empty((B, T, C), dtype=np.float32)
    for b in range(B):
        out[b] = (res.results[2 * b]["out"].astype(np.float32)
                  + res.results[2 * b + 1]["out"].astype(np.float32) + b_proj)
    if _run_kwargs:
        kernel.last_results = res
    return out
